# revision 38
# baseline (speedup 1.0000x reference)
"""Multi-head attention block (b=8, n=1024, d=1024, heads=16) on 8 trn2
NeuronCores, data-parallel over batch (one batch element per core).

Matmul operands are bf16 (PE streams 1 col/cycle; fp32 is 4 cycles/col).
PSUM accumulation and softmax math stay fp32.

Per-core dataflow (all matmuls on PE):
  B:  qkT[c, t]  = sum_d WqkvT[d, c] * xT[d, t]      (q,k channels 0..2047)
  C:  V[t, c]    = sum_d xT[d, t]    * WqkvT[d, 2048+c]
  D:  per HEAD PAIR (the two K=64 S^T matmuls run on PE row groups
      0-63 / 64-127 into the two column halves of a shared [128,1024]
      PSUM tile, so one exp covers both heads):
        S^T[j, i] = sum_d kT[d, j] qT[d, i]           (K=64 matmul)
        E = exp(S^T * scale)                          (ACT, no max-subtract:
                                                       |scores*scale| < ~3)
        [O^T_u; rowsum] = [V_h | 1]^T E               (ones column appended to
                                                       V gives rowsum for free)
        O^T = O^T_u / rowsum                          (DMA partition-broadcast
                                                       of the raw rowsum row +
                                                       DVE divide -- see below)
  E:  yT[o, t] = sum_D WprojT[D, o] O^T[D, t] + bias[o]

Why the softmax normalization uses DMA + divide: the pair loop is
co-limited by ACT (16 exps = ~17us/pair) and PE (S^T+AV+B = ~20.5us),
coupled at fine grain through the two-deep S^T PSUM pool. Any extra op on
ACT (the old exp(-ln x) reciprocal) delays the exp stream and stalls the
PE via that pool; any PE-side broadcast matmul allocates from the same
pool and chains the next pair's S^T to this pair's normalization. Instead:
the rowsum row (PSUM partition 64 of each O^T_u tile) is replicated to 64
partitions by a DMA with a 0-stride partition source AP (the DMA engines
are otherwise idle mid-pair), and one DVE tensor_tensor(divide) per half
normalizes straight out of PSUM. ACT runs pure exps; nothing else touches
the S^T pool.

Other schedule notes:
- The next pair's two B chains run AFTER the j-loop (PE filler that
  covers the normalization chain), and their PSUM->SBUF copies are split
  per column half between ACT and DVE so the next pair's first S^T waits
  ~0.6us, not ~1.1us, for its q/k tile.
- AV runs three j-steps behind S^T so the previous pair's divides (DVE)
  and rowsum-broadcast DMA reads can release/read the psO banks before
  this pair's start=True write reuses them.
- x^T is loaded as 8 separate 128-row tiles with DMAs issued from three
  engine queues: the first B matmul starts after one chunk lands.
- W_proj prefetches via the sync queue right after stage C; B weights
  prefetch one pair ahead.
- yT is written bf16 (the weights are already bf16; end-to-end error
  ~4e-3 vs the 2e-2 gate): halves the output-drain tail. Full-width
  [128,1024] writes keep DMA rows contiguous (2KB).

Odd heads land on SBUF partitions 64..127 of the O^T tile via a
SBUF->SBUF DMA (DVE lanes are partition-local and cannot shift
partitions).

Host does only data movement: transposes / tiling rearranges of x and the
weights (cast to bf16), and the inverse transpose of the output.
"""

import json

import ml_dtypes
import numpy as np

D = 1024
NT = 1024
H = 16
HD = 64
P = 128
DC = D // P  # 8 contraction chunks
SCALE = HD ** -0.5
N_CORES = 8

_CACHE = {}


# --------------------------------------------------------------------------
# Workaround for the walrus build in this container: each TPB instruction
# encodes at most ONE sync wait (NEURON_ISA_TPB_EVENTS has a single wait
# slot) and this walrus version errors out instead of splitting. Tile
# attaches several waits per instruction. Hoist all but the last wait onto
# preceding single-wait EventSemaphore no-ops on the same (in-order) engine.
# --------------------------------------------------------------------------
def _split_sync_waits_json(bir_bytes: bytes) -> bytes:
    j = json.loads(bir_bytes)
    changed = False
    ctr = 0
    dma_ops = {"TensorLoad", "TensorSave", "TensorCopy", "TensorReduce"}
    for fn in j.get("functions", []):
        for blk in fn.get("blocks", []):
            out = []
            for inst in blk.get("instructions", []):
                si = inst.get("sync_info")
                if si:
                    waits = si.get("on_wait") or []
                    if len(waits) > 1:
                        for w in waits[:-1]:
                            ctr += 1
                            out.append(
                                {
                                    "debug": inst.get("debug", 0),
                                    "engine": inst.get("engine"),
                                    "ins": [],
                                    "outs": [],
                                    "name": f"splitw-{ctr}-{inst['name']}",
                                    "opcode": "EventSemaphore",
                                    "sync_info": {"on_update": [], "on_wait": [w]},
                                }
                            )
                        si["on_wait"] = [waits[-1]]
                        changed = True
                    ups = si.get("on_update") or []
                    if len(ups) > 1 and inst.get("opcode") not in dma_ops:
                        extra = ups[:-1]
                        si["on_update"] = [ups[-1]]
                        out.append(inst)
                        for u in extra:
                            ctr += 1
                            out.append(
                                {
                                    "debug": inst.get("debug", 0),
                                    "engine": inst.get("engine"),
                                    "ins": [],
                                    "outs": [],
                                    "name": f"splitu-{ctr}-{inst['name']}",
                                    "opcode": "EventSemaphore",
                                    "sync_info": {"on_update": [u], "on_wait": []},
                                }
                            )
                        changed = True
                        continue
                out.append(inst)
            blk["instructions"] = out
    if not changed:
        return bir_bytes
    return json.dumps(j).encode()


def _install_bir_fix():
    import concourse.bass as bass

    if getattr(bass.Bass, "_split_waits_patched", False):
        return
    orig = bass.Bass.to_json_bytes

    def patched(self, *a, **kw):
        return _split_sync_waits_json(orig(self, *a, **kw))

    bass.Bass.to_json_bytes = patched
    bass.Bass._split_waits_patched = True


def _build_module():
    from contextlib import ExitStack

    import concourse.bass as bass
    import concourse.tile as tile
    from concourse import mybir

    _install_bir_fix()
    f32 = mybir.dt.float32
    bf16 = mybir.dt.bfloat16
    nc = bass.Bass(num_swdge_queues=4)

    xT = nc.declare_dram_parameter("xT", [D, NT], bf16, isOutput=False)
    # wqk[p, ct, a, c] = W_qkv.T[a*128+p, ct*128+c]  (q,k channels, ct<16)
    wqk = nc.declare_dram_parameter("wqk", [P, 16, DC, P], bf16, isOutput=False)
    # wv[p, a, cv] = W_qkv.T[a*128+p, 2048+cv]
    wvp = nc.declare_dram_parameter("wv", [P, DC, D], bf16, isOutput=False)
    # wpr[p, ot, a, c] = W_proj.T[a*128+p, ot*128+c]
    wpr = nc.declare_dram_parameter("wpr", [P, DC, DC, P], bf16, isOutput=False)
    # biasT[p, t] = b_proj[t*128+p]
    biasT = nc.declare_dram_parameter("biasT", [P, DC], f32, isOutput=False)
    yT = nc.declare_dram_parameter("yT", [D, NT], bf16, isOutput=True)

    with tile.TileContext(nc) as tc, ExitStack() as outer:
        v_pool = outer.enter_context(tc.tile_pool(name="vsb", bufs=1))
        ot_pool = outer.enter_context(tc.tile_pool(name="otp", bufs=1))
        qk_pool = outer.enter_context(tc.tile_pool(name="qkp", bufs=2))
        misc = outer.enter_context(tc.tile_pool(name="misc", bufs=1))
        xt_pool = outer.enter_context(tc.tile_pool(name="xt", bufs=1))
        wt_pool = outer.enter_context(tc.tile_pool(name="wt", bufs=6))
        wp_pool = outer.enter_context(tc.tile_pool(name="wp", bufs=1))

        # --- input DMAs first: the first B matmul needs only wt0 + x chunk
        # 0. Spread descriptor generation across the three DMA-capable
        # engine queues (~650ns per dma_start on the issuing engine).
        wt0 = wt_pool.tile([P, DC, P], bf16, tag="wt")
        nc.gpsimd.dma_start(wt0[:], wqk[:, 0, :, :])
        xts = []
        iss = [nc.sync, nc.scalar, nc.gpsimd]
        for a in range(DC):
            t = xt_pool.tile([P, NT], bf16, tag=f"x{a}", name=f"xt{a}")
            xts.append(t)
            iss[a % 3].dma_start(t[:], xT[a * P : (a + 1) * P, :])
        wt8 = wt_pool.tile([P, DC, P], bf16, tag="wt")
        nc.gpsimd.dma_start(wt8[:], wqk[:, 8, :, :])

        v_sb = v_pool.tile([P, DC, H, HD + 1], bf16)  # V + ones column per head
        ot = ot_pool.tile([P, DC, NT], bf16)          # O^T, channel-major
        ones_f = misc.tile([P, HD], f32)
        ones_t = misc.tile([P, HD], bf16)
        bias_t = misc.tile([P, DC], f32)
        nc.vector.memset(ones_f[:], 1.0)
        nc.vector.tensor_copy(ones_t[:], ones_f[:])
        nc.sync.dma_start(bias_t[:], biasT[:])
        for vt in range(DC):
            nc.vector.tensor_copy(v_sb[:, vt, :, HD], ones_f[:, 0:H])

        wpt_all = wp_pool.tile([P, DC, DC, P], bf16)

        # ------- stages B+D interleaved: qk projection + attention -------
        with (
            tc.tile_pool(name="es", bufs=20) as es_pool,
            tc.tile_pool(name="tmp", bufs=2) as tmp_pool,
            tc.tile_pool(name="rbp", bufs=2) as rb_pool,
            tc.tile_pool(name="psS", bufs=2, space="PSUM") as psS,
            tc.tile_pool(name="psO", bufs=1, space="PSUM") as psO,
        ):

            def emit_b(ct, wt):
                # qkT[c, t] for one 128-channel tile (2 heads' q or k)
                ps = psS.tile([P, NT], f32, tag="sps")
                for a in range(DC):
                    for nh in range(2):
                        nc.tensor.matmul(
                            ps[:, nh * 512 : (nh + 1) * 512],
                            wt[:, a, :],
                            xts[a][:, nh * 512 : (nh + 1) * 512],
                            start=(a == 0),
                            stop=(a == DC - 1),
                        )
                if ct < 8:
                    t = qk_pool.tile([P, NT], bf16, tag="qt")
                else:
                    t = qk_pool.tile([P, NT], bf16, tag="kt")
                # both halves on the DVE (ACT's exp stream is the pair
                # loop's tightest queue); two instructions so the next
                # pair's first S^T tiles can start after the first half
                nc.vector.tensor_copy(t[:, 0:512], ps[:, 0:512])
                nc.vector.tensor_copy(t[:, 512:NT], ps[:, 512:NT])
                return t

            def wt_fetch(ct):
                wt = wt_pool.tile([P, DC, P], bf16, tag="wt")
                nc.gpsimd.dma_start(wt[:], wqk[:, ct, :, :])
                return wt

            def emit_b_qt_early(ct, wt, kt_n):
                # build the next pair's q tile with the pair's first two
                # S^T(ih=0) tiles woven between the two column-half
                # accumulation chains. Their exps fill the only window
                # where ACT's exp stream idles (the B chains), and the
                # next pair's paced j-loop then has 14 exps, not 16. The
                # S^T PSUM is borrowed from the psO slots the O^T_u
                # copies just released (the psS ring is fully booked by
                # the j-loop and the two B chains).
                ps = psS.tile([P, NT], f32, tag="sps")
                for a in range(DC):
                    nc.tensor.matmul(
                        ps[:, 0:512],
                        wt[:, a, :],
                        xts[a][:, 0:512],
                        start=(a == 0),
                        stop=(a == DC - 1),
                    )
                t = qk_pool.tile([P, NT], bf16, tag="qt")
                nc.vector.tensor_copy(t[:, 0:512], ps[:, 0:512])
                es_early = []
                for j in range(2):
                    esj = es_pool.tile([P, NT], bf16, name="es")
                    for half, tagn in ((0, "opA0"), (1, "opA1")):
                        sps = psO.tile([P, 512], f32, tag=tagn)
                        nc.tensor.matmul(
                            sps[:],
                            kt_n[half * HD : (half + 1) * HD, j * P : (j + 1) * P],
                            t[half * HD : (half + 1) * HD, 0:512],
                            start=True,
                            stop=True,
                        )
                        nc.scalar.activation(
                            esj[:, half * 512 : (half + 1) * 512], sps[:],
                            mybir.ActivationFunctionType.Exp, scale=SCALE,
                        )
                    es_early.append(esj)
                for a in range(DC):
                    nc.tensor.matmul(
                        ps[:, 512:NT],
                        wt[:, a, :],
                        xts[a][:, 512:NT],
                        start=(a == 0),
                        stop=(a == DC - 1),
                    )
                nc.vector.tensor_copy(t[:, 512:NT], ps[:, 512:NT])
                return t, es_early

            def norm_full(h, ob):
                # softmax normalization off the PE and at minimal ACT
                # cost, scheduled under the B-chain window (the only time
                # ACT's exp stream has slack):
                #  1. the O^T_u + rowsum copies (pair_block end, DVE)
                #     already released the psO banks and put the data in
                #     SBUF (ob, [65, 1024] per head);
                #  2. 1/rowsum = exp(-ln x) on ACT: ONE Ln + ONE Exp over
                #     the [1,1024] row per head (batched: ~2us/pair, vs
                #     ~43us total for the old 64 tiny per-row ops);
                #  3. a K=1 PE outer product replicates the reciprocal
                #     row to 64 partitions, writing into this head's just-
                #     freed psO slot -- NOT the S^T pool, so the next
                #     pair's S^T stream is never chained to this (a DMA
                #     broadcast was tried instead: SBUF->SBUF DMA runs at
                #     ~28 GB/s and the 512KB/pair of replication traffic
                #     swamped the pair tail);
                #  4. one DVE multiply per half (ob SBUF x bps PSUM -> one
                #     PSUM operand only, which is legal) normalizes.
                odd = h % 2 == 1
                tag = "B" if odd else "A"
                rln = rb_pool.tile([HD + 1, NT], f32, tag="rln", bufs=2)
                rsr = rb_pool.tile([HD + 1, NT], bf16, tag="rsr", bufs=2)
                nc.scalar.activation(
                    rln[HD : HD + 1, :], ob[HD : HD + 1, :],
                    mybir.ActivationFunctionType.Ln,
                )
                nc.scalar.activation(
                    rsr[HD : HD + 1, :], rln[HD : HD + 1, :],
                    mybir.ActivationFunctionType.Exp, scale=-1.0,
                )
                if odd:
                    dst = tmp_pool.tile([HD, NT], bf16)
                else:
                    dst = ot[0:HD, h // 2, :]
                for ih in range(2):
                    # K=1 outer-product broadcast into this head's just-
                    # freed psO slot (the ob copy released it); the
                    # multiply reads ob (SBUF) x bps (PSUM) directly
                    bps = psO.tile([HD, 512], f32, tag=f"op{tag}{ih}")
                    nc.tensor.matmul(
                        bps[:],
                        ones_t[HD : HD + 1, :],
                        rsr[HD : HD + 1, ih * 512 : (ih + 1) * 512],
                        start=True,
                        stop=True,
                    )
                    nc.vector.tensor_mul(
                        dst[:, ih * 512 : (ih + 1) * 512],
                        ob[0:HD, ih * 512 : (ih + 1) * 512],
                        bps[:],
                    )
                if odd:
                    # lanes cannot shift partitions; DMA moves the odd
                    # head's rows to partitions 64..127
                    nc.gpsimd.dma_start(ot[HD:P, h // 2, :], dst[:])

            # process heads in PAIRS: the two heads' K=64 S^T matmuls run
            # on PE row groups 0-63 / 64-127 (row tiling), into the two
            # column halves of a shared [P, 1024] PSUM tile, so one exp
            # covers both heads.
            def pair_block(hp, qt, kt, es_pre=None, es_early=None):
                hA, hB = 2 * hp, 2 * hp + 1
                qsA, ksA = qt[0:HD, :], kt[0:HD, :]
                qsB, ksB = qt[HD:P, :], kt[HD:P, :]
                es_list = [None] * DC
                opA = opB = None

                def emit_st(j):
                    # ih=0 for j<2 may have been computed early (during
                    # the previous pair's B chains)
                    if es_early is not None and j < 2:
                        out = [es_early[j]]
                        ihs = (1,)
                    else:
                        out = []
                        ihs = (0, 1)
                    for ih in ihs:
                        sps = psS.tile([P, NT], f32, tag="sps")
                        for qs, ks, half in ((qsA, ksA, 0), (qsB, ksB, 1)):
                            nc.tensor.matmul(
                                sps[:, half * 512 : (half + 1) * 512],
                                ks[:, j * P : (j + 1) * P],
                                qs[:, ih * 512 : (ih + 1) * 512],
                                start=True,
                                stop=True,
                            )
                        es = es_pool.tile([P, NT], bf16)
                        nc.scalar.activation(
                            es[:], sps[:], mybir.ActivationFunctionType.Exp,
                            scale=SCALE,
                        )
                        out.append(es)
                    return tuple(out)

                def do_av(j):
                    nonlocal opA, opB
                    if opA is None:
                        opA0 = psO.tile([P, 512], f32, tag="opA0")
                        opA1 = psO.tile([P, 512], f32, tag="opA1")
                        opB0 = psO.tile([P, 512], f32, tag="opB0")
                        opB1 = psO.tile([P, 512], f32, tag="opB1")
                        opA = (opA0, opA1)
                        opB = (opB0, opB1)
                    for ih in range(2):
                        for half, h, ops in ((0, hA, opA), (1, hB, opB)):
                            nc.tensor.matmul(
                                ops[ih][0 : HD + 1, :],
                                v_sb[:, j, h, :],
                                es_list[j][ih][:, half * 512 : (half + 1) * 512],
                                start=(j == 0),
                                stop=(j == DC - 1),
                            )

                for j in range(DC):
                    es_list[j] = es_pre[j] if es_pre is not None else emit_st(j)
                    # AV two steps behind S^T: the previous pair's O^T_u
                    # copies (which read and thereby release the psO
                    # banks) get time to finish before this pair's
                    # start=True write reuses them
                    if j >= 2:
                        do_av(j - 2)
                do_av(DC - 2)
                do_av(DC - 1)
                # copy O^T_u + rowsum row to SBUF right away (DVE): this
                # is what releases the psO banks for the next pair, and
                # norm_full then runs entirely out of SBUF
                obA = rb_pool.tile([HD + 1, NT], f32, tag="obA", bufs=2)
                obB = rb_pool.tile([HD + 1, NT], f32, tag="obB", bufs=2)
                for ih in range(2):
                    nc.vector.tensor_copy(
                        obA[:, ih * 512 : (ih + 1) * 512], opA[ih][0 : HD + 1, :]
                    )
                    nc.vector.tensor_copy(
                        obB[:, ih * 512 : (ih + 1) * 512], opB[ih][0 : HD + 1, :]
                    )
                return ((hA, obA), (hB, obB))

            # ---- stage C (V = x @ Wv^T), woven with pair 0's S^T/exp ----
            qt = emit_b(0, wt0)
            kt = emit_b(8, wt8)
            es0 = [None] * DC
            with tc.tile_pool(name="wvt", bufs=1) as wv_pool:
                # per-chunk loads on alternating queues: C's first matmul
                # needs only chunk a=0, and the 2 MB no longer contends
                # with the tail of the x^T stream as one monolithic burst
                wvs = []
                for a in range(DC):
                    wva = wv_pool.tile([P, D], bf16, tag=f"wv{a}", name=f"wv{a}")
                    wvs.append(wva)
                    iss[a % 3].dma_start(wva[:], wvp[:, a, :])
                for vt in range(DC):
                    # alternate across all four psO tags so consecutive vt
                    # iterations double-buffer (each tag has bufs=1)
                    if vt % 2 == 0:
                        pv0 = psO.tile([P, 512], f32, tag="opA0")
                        pv1 = psO.tile([P, 512], f32, tag="opA1")
                    else:
                        pv0 = psO.tile([P, 512], f32, tag="opB0")
                        pv1 = psO.tile([P, 512], f32, tag="opB1")
                    for a in range(DC):
                        for ch, ps in ((0, pv0), (1, pv1)):
                            nc.tensor.matmul(
                                ps[:],
                                xts[a][:, vt * P : (vt + 1) * P],
                                wvs[a][:, ch * 512 : (ch + 1) * 512],
                                start=(a == 0),
                                stop=(a == DC - 1),
                            )
                    # weave pair 0's S^T so ACT starts its exps early
                    j = vt
                    for ih in range(2):
                        sps = psS.tile([P, NT], f32, tag="sps")
                        for qo2 in (0, HD):
                            nc.tensor.matmul(
                                sps[:, (qo2 // HD) * 512 : (qo2 // HD + 1) * 512],
                                kt[qo2 : qo2 + HD, j * P : (j + 1) * P],
                                qt[qo2 : qo2 + HD, ih * 512 : (ih + 1) * 512],
                                start=True,
                                stop=True,
                            )
                        es = es_pool.tile([P, NT], bf16)
                        nc.scalar.activation(
                            es[:], sps[:], mybir.ActivationFunctionType.Exp,
                            scale=SCALE,
                        )
                        if es0[j] is None:
                            es0[j] = [None, None]
                        es0[j][ih] = es
                    for ch, ps in ((0, pv0), (1, pv1)):
                        # one strided copy per half (dst skips each head's
                        # ones column) instead of 8 small copies: same
                        # bytes, 1/8th the DVE instruction overhead
                        nc.vector.tensor_copy(
                            v_sb[:, vt, ch * 8 : (ch + 1) * 8, 0:HD],
                            ps[:].rearrange("p (h d) -> p h d", h=8),
                        )
            es0 = [tuple(e) for e in es0]

            # prefetch the 2 MB of proj weights now, from the sync queue:
            # they land during the pair loop while the DMA engines idle
            for oi in range(DC):
                nc.sync.dma_start(wpt_all[:, oi, :, :], wpr[:, oi, :, :])

            # B weights for the next pair prefetch one pair ahead
            nwt = (wt_fetch(1), wt_fetch(9))
            nes = None
            for hp in range(8):
                cwt, nwt = nwt, None
                if hp + 2 < 8:
                    nwt = (wt_fetch(hp + 2), wt_fetch(10 + hp))
                res = pair_block(
                    hp, qt, kt,
                    es_pre=es0 if hp == 0 else None,
                    es_early=nes,
                )
                nes = None
                if hp + 1 < 8:
                    # k chain first so its SBUF tile (the S^T stationary)
                    # is ready for the early S^T tiles and for the next
                    # pair's loop with minimal latency
                    kt = emit_b(9 + hp, cwt[1])
                    qt, nes = emit_b_qt_early(hp + 1, cwt[0], kt)
                for entry in res:
                    norm_full(*entry)

        # -------- stage E: output projection + bias --------
        with (
            tc.tile_pool(name="outp", bufs=3) as out_pool,
            tc.tile_pool(name="psE", bufs=2, space="PSUM") as psE,
        ):
            for oi in range(DC):
                wpt = wpt_all[:, oi, :, :]
                osb = out_pool.tile([P, NT], bf16)
                pe = psE.tile([P, NT], f32, tag="pse")
                for a in range(DC):
                    for nh in range(2):
                        nc.tensor.matmul(
                            pe[:, nh * 512 : (nh + 1) * 512],
                            wpt[:, a, :],
                            ot[:, a, nh * 512 : (nh + 1) * 512],
                            start=(a == 0),
                            stop=(a == DC - 1),
                        )
                if oi < DC - 1:
                    nc.vector.tensor_scalar_add(
                        osb[:], pe[:], bias_t[:, oi : oi + 1]
                    )
                    nc.gpsimd.dma_start(yT[oi * P : (oi + 1) * P, :], osb[:])
                else:
                    # last chunk: bias + DMA per half so the final exposed
                    # transfer is half the size
                    for nh in range(2):
                        nc.vector.tensor_scalar_add(
                            osb[:, nh * 512 : (nh + 1) * 512],
                            pe[:, nh * 512 : (nh + 1) * 512],
                            bias_t[:, oi : oi + 1],
                        )
                        nc.gpsimd.dma_start(
                            yT[oi * P : (oi + 1) * P, nh * 512 : (nh + 1) * 512],
                            osb[:, nh * 512 : (nh + 1) * 512],
                        )

    return nc


def _get_nc():
    if "nc" not in _CACHE:
        _CACHE["nc"] = _build_module()
    return _CACHE["nc"]


def _host_inputs(x, W_qkv, W_proj, b_proj):
    bf = ml_dtypes.bfloat16
    x = np.asarray(x, dtype=np.float32).astype(bf)
    W_qkv = np.asarray(W_qkv, dtype=np.float32).astype(bf)
    W_proj = np.asarray(W_proj, dtype=np.float32).astype(bf)
    b_proj = np.asarray(b_proj, dtype=np.float32)

    wqkvT = W_qkv.T  # [1024, 3072]
    # wqk[p, ct, a, c] = wqkvT[a*128+p, ct*128+c] for q,k channels
    wqk = np.ascontiguousarray(
        wqkvT[:, : 2 * D].reshape(DC, P, 16, P).transpose(1, 2, 0, 3)
    )
    # wv[p, a, cv] = wqkvT[a*128+p, 2048+cv]
    wv = np.ascontiguousarray(wqkvT[:, 2 * D :].reshape(DC, P, D).transpose(1, 0, 2))
    # wpr[p, ot, a, c] = W_proj.T[a*128+p, ot*128+c]
    wpr = np.ascontiguousarray(
        W_proj.T.reshape(DC, P, DC, P).transpose(1, 2, 0, 3)
    )
    biasT = np.ascontiguousarray(b_proj.reshape(DC, P).T)

    in_maps = []
    for i in range(N_CORES):
        in_maps.append(
            {
                "xT": np.ascontiguousarray(x[i].T),
                "wqk": wqk,
                "wv": wv,
                "wpr": wpr,
                "biasT": biasT,
            }
        )
    return in_maps


def _run(in_maps, trace=False):
    from concourse.bass_utils import run_bass_kernel_spmd

    nc = _get_nc()
    return run_bass_kernel_spmd(nc, in_maps, list(range(N_CORES)), trace=trace)


def kernel(x, W_qkv, W_proj, b_proj):
    in_maps = _host_inputs(x, W_qkv, W_proj, b_proj)
    res = _run(in_maps)
    out = np.stack(
        [res.results[i]["yT"].T.astype(np.float32) for i in range(N_CORES)], axis=0
    )
    return np.ascontiguousarray(out)
